# revision 2
# baseline (speedup 1.0000x reference)
"""AGCNN_IA Trainium2 kernel v2: 3x self-att + 1x cross-att + 5x conv-BN-lrelu.

Sharding: data-parallel over batch B=8 across 8 NeuronCores (1 sample/core).
BN batch statistics are AllReduce'd (sum, sumsq per channel) across cores.

v2 redesign vs v1 (1.31 ms):
  - features kept in fp16 everywhere (prelu writes fp16); self-att scores are
    computed EXACTLY over the fp16 feature values by a single fp16 matmul
    (fp16 products accumulate in fp32 on the PE), with the -xx[m] column term
    riding as two extra fp16 rows (hi/lo pair) of the rhs against ones rows
    in lhsT. This replaces v1's 4-cycle fp32 matmuls (4x PE reduction).
  - top-20 via 16 segment max8 (top-8 per 128-wide segment -> 128 candidates)
    + 3 max8/2 match_replace on the candidates. Replaces 5 full-width DVE
    passes (11.4 us/chunk) with ~1.2 full passes (~5 us/chunk).
  - threshold mask (score >= t20) evaluated on exact fp32 scores via one DVE
    scalar_tensor_tensor; weights = mask * exp(score - mx - ln z20).
  - convs all fp16 (weights transposed to fp16), BN stats fp32 accum.
"""

import sys

import numpy as np

sys.path.insert(0, "/opt/trn_rl_repo")

B = 8
N = 2048
KTOP = 20
EPS_BN = 1e-5
NCH = N // 128  # 16 row chunks
NB = N // 512  # 4 matmul free-dim blocks
NSEG = 16  # topk segments per row chunk
SEGW = N // NSEG
BN_CNT = float(B * N)

_CACHE = {}


def _patch_act_tables():
    """Force every ACT function onto the one table set that has them all."""
    from concourse import bacc

    orig = bacc.get_activation_tables
    if getattr(orig, "_patched_single_set", False):
        return

    def patched(module_arch):
        tables = orig(module_arch)
        keep = "natural_log_exp_and_others"
        if keep in tables:
            tables = {
                name: (fns if name == keep else set())
                for name, fns in tables.items()
            }
        return tables

    patched._patched_single_set = True
    bacc.get_activation_tables = patched


def _build():
    import concourse.bass as bass
    import concourse.mybir as mybir
    from concourse import bacc, tile
    from concourse.masks import make_identity

    _patch_act_tables()

    dt = mybir.dt.float32
    fp16 = mybir.dt.float16
    AF = mybir.ActivationFunctionType
    ALU = mybir.AluOpType
    AX = mybir.AxisListType

    nc = bacc.Bacc(None, target_bir_lowering=False, debug=False, num_devices=8)

    x_d = nc.declare_dram_parameter("x", [3, N], dt, isOutput=False)
    y3_d = nc.declare_dram_parameter("y3", [128, N], dt, isOutput=False)
    w_d = {}
    g_d = {}
    b_d = {}
    convs = {1: (6, 64), 2: (128, 64), 3: (128, 128), 4: (256, 256), 5: (512, 512)}
    for i, (ci, co) in convs.items():
        w_d[i] = nc.declare_dram_parameter(f"w{i}", [co, ci], dt, isOutput=False)
        g_d[i] = nc.declare_dram_parameter(f"g{i}", [co], dt, isOutput=False)
        b_d[i] = nc.declare_dram_parameter(f"b{i}", [co], dt, isOutput=False)
    out_d = nc.declare_dram_parameter("out", [512, N], dt, isOutput=True)

    with tile.TileContext(nc) as tc:
        with (
            tc.tile_pool(name="persist", bufs=1) as persist,
            tc.tile_pool(name="scratch", bufs=1) as scratch_pool,
            tc.tile_pool(name="dram", bufs=1, space="DRAM") as dram,
        ):
            ident = persist.tile([128, 128], dt)
            make_identity(nc, ident[:, :])
            ident_h = persist.tile([128, 128], fp16)
            make_identity(nc, ident_h[:, :])
            ones_col_h = persist.tile([128, 1], fp16)
            nc.vector.memset(ones_col_h[:, :], 1.0)
            ones2h = persist.tile([2, N], fp16)
            nc.vector.memset(ones2h[:, :], 1.0)
            eps_t = persist.tile([128, 1], dt)
            nc.vector.memset(eps_t[:, :], EPS_BN)
            # xc: concat buffer [128, 4, N] fp16; ch c = xc[c%128, c//128, :]
            xc = persist.tile([128, 4, N], fp16)
            # attention in/out feature tiles (fp16): attR holds [x ; negxx pair]
            # L1 (3-D points) is fp32: NN selection there is hypersensitive to
            # coordinate rounding, so scores/xx stay exact fp32.
            attR1 = persist.tile([4, N], dt)
            attL1 = persist.tile([4, N], dt)
            xh1 = persist.tile([3, N], fp16)
            ones_row32 = persist.tile([1, N], dt)
            nc.vector.memset(ones_row32[:, :], 1.0)
            ones_col32 = persist.tile([128, 1], dt)
            nc.vector.memset(ones_col32[:, :], 1.0)
            scratch32 = persist.tile([128, N], dt)
            attR2 = persist.tile([66, N], fp16)
            attL2 = persist.tile([66, N], fp16)
            attR3 = persist.tile([66, N], fp16)
            attL3 = persist.tile([66, N], fp16)
            # fp16 elementwise scratch (squares for sumsq, conv square dump)
            scratch_h = scratch_pool.tile([128, N], fp16)
            xxrow = persist.tile([1, N], dt)
            negxx32 = persist.tile([1, N], dt)
            pair_hi = persist.tile([1, N], fp16)
            pair_lo = persist.tile([1, N], fp16)

            # ---------- conv weight prep (no data deps; scheduler overlaps) --
            wt_sb = {}
            with (
                tc.tile_pool(name="wprep", bufs=1) as wp,
                tc.tile_pool(name="wprep_ps", bufs=2, space="PSUM") as wps,
            ):
                chunk_splits = {
                    1: [3, 3],
                    2: [64, 64],
                    3: [64, 64],
                    4: [128, 128],
                    5: [128, 128, 128, 128],
                }
                for li, (C_in, C_out) in convs.items():
                    coP = min(C_out, 128)
                    CO = (C_out + 127) // 128
                    splits = chunk_splits[li]
                    w_sb = wp.tile([coP, CO, C_in], dt, tag=f"w_sb{li}")
                    nc.sync.dma_start(
                        w_sb[:, :, :],
                        w_d[li].ap().rearrange("(a p) c -> p a c", p=coP),
                    )
                    wt = persist.tile(
                        [128, len(splits), CO, coP], fp16, tag=f"wt{li}"
                    )
                    off = 0
                    for kc, Ck in enumerate(splits):
                        for co in range(CO):
                            ps = wps.tile([128, 128], dt, tag="wtps")
                            nc.tensor.transpose(
                                ps[0:Ck, 0:coP],
                                w_sb[0:coP, co, off : off + Ck],
                                ident[0:coP, 0:coP],
                            )
                            nc.scalar.copy(wt[0:Ck, kc, co, :], ps[0:Ck, 0:coP])
                        off += Ck
                    wt_sb[li] = wt

            def col_sumsq_row(src_ap, C, dst_row_ap):
                """dst_row[0, m] = sum_c src[c, m]^2 (fp32 squares + fp32 PE)."""
                nc.scalar.activation(scratch32[0:C, :], src_ap, AF.Square)
                with tc.tile_pool(name="psx", bufs=2, space="PSUM") as psx:
                    for nb in range(NB):
                        ps = psx.tile([1, 512], dt)
                        nc.tensor.matmul(
                            ps[:, :],
                            ones_col32[0:C, :],
                            scratch32[0:C, nb * 512 : (nb + 1) * 512],
                            start=True,
                            stop=True,
                        )
                        nc.scalar.copy(
                            dst_row_ap[0:1, nb * 512 : (nb + 1) * 512], ps[:, :]
                        )

            def make_negxx_pair(src_ap, C, dst_rows_ap):
                """attR aug rows: fp16 hi/lo pair of -xx, DMA'd into place."""
                col_sumsq_row(src_ap, C, xxrow)
                nc.scalar.activation(
                    pair_hi[0:1, :], xxrow[0:1, :], AF.Copy, scale=-1.0
                )
                # lo = (-xx) - hi
                nc.vector.scalar_tensor_tensor(
                    pair_lo[0:1, :],
                    xxrow[0:1, :],
                    -1.0,
                    pair_hi[0:1, :],
                    op0=ALU.mult,
                    op1=ALU.subtract,
                )
                nc.sync.dma_start(dst_rows_ap[0:1, :], pair_hi[0:1, :])
                nc.sync.dma_start(dst_rows_ap[1:2, :], pair_lo[0:1, :])

            def transpose_to(src_ap, C, dst_tile):
                """src [C, N] fp16 -> dst [128, NCH, >=C] fp16."""
                Cp = C + (C % 2)  # 4B-aligned PSUM rows for fp16
                with tc.tile_pool(name="pst", bufs=2, space="PSUM") as pst:
                    for t0 in range(0, NCH, 4):
                        ps = pst.tile([128, 4, Cp], fp16)
                        for t in range(t0, t0 + 4):
                            nc.tensor.transpose(
                                ps[:, t - t0, 0:C],
                                src_ap[:, t * 128 : (t + 1) * 128],
                                ident_h[0:C, 0:C],
                            )
                        nc.scalar.copy(
                            dst_tile[:, t0 : t0 + 4, 0:C], ps[:, :, 0:C]
                        )

            def attention(lhsT_ap, K_dim, Cf, fT, f_src, rnx_col, get_rhs, agg_dst):
                """Top-k attention. agg_dst [Cf, N] fp16 = W @ f - f_src.

                lhsT_ap(isl) -> [K_dim, 128] fp16 stationary for score matmul
                get_rhs(nb) -> [K_dim, 512] fp16 moving
                fT: [128, NCH, Cf] fp16 transposed gather-source features
                f_src: [Cf, N] fp16 subtract source
                rnx_col: [128, NCH] per-row 1/||x_n|| or None
                """
                with (
                    tc.tile_pool(name="att_sb", bufs=2) as att_sb,
                    tc.tile_pool(name="att_w", bufs=2) as att_w,
                    tc.tile_pool(name="att_small", bufs=2) as small,
                    tc.tile_pool(name="ps_g", bufs=1, space="PSUM") as ps_g,
                    tc.tile_pool(name="ps_a", bufs=2, space="PSUM") as ps_a,
                ):
                    GRP = 2
                    for g in range(NCH // GRP):  # groups of row-chunks
                        wt4 = att_w.tile([128, NCH, GRP, 128], fp16, tag="wt4")
                        for s in range(GRP):  # sub-chunk within group
                            i = g * GRP + s
                            isl = slice(i * 128, (i + 1) * 128)
                            score = att_sb.tile([128, N], dt, tag="score")
                            ps = ps_g.tile([128, NB, 512], dt, tag="ps")
                            for nb in range(NB):
                                nc.tensor.matmul(
                                    ps[:, nb, :],
                                    lhsT_ap(isl),
                                    get_rhs(nb),
                                    start=True,
                                    stop=True,
                                )
                                if rnx_col is None:
                                    nc.scalar.copy(
                                        score[:, nb * 512 : (nb + 1) * 512],
                                        ps[:, nb, :],
                                    )
                                else:
                                    nc.scalar.activation(
                                        score[:, nb * 512 : (nb + 1) * 512],
                                        ps[:, nb, :],
                                        AF.Copy,
                                        scale=rnx_col[:, i : i + 1],
                                    )
                            # segment top-8 candidates (exact unless one
                            # segment holds >8 of the true top-20: ~1e-5)
                            cand = small.tile([128, NSEG * 8], dt, tag="cand")
                            for sg in range(NSEG):
                                nc.vector.max(
                                    cand[:, sg * 8 : (sg + 1) * 8],
                                    score[:, sg * SEGW : (sg + 1) * SEGW],
                                )
                            t24 = small.tile([128, 32], dt, tag="t24")
                            candm = small.tile([128, NSEG * 8], dt, tag="candm")
                            nc.vector.max(t24[:, 0:8], cand[:, :])
                            nc.vector.match_replace(
                                candm[:, :], t24[:, 0:8], cand[:, :], -1e30
                            )
                            nc.vector.max(t24[:, 8:16], candm[:, :])
                            nc.vector.match_replace(
                                candm[:, :], t24[:, 8:16], candm[:, :], -1e30
                            )
                            nc.vector.max(t24[:, 16:24], candm[:, :])
                            th = t24[:, 19:20]
                            # bias2 = -(row_max + ln z20)
                            nth = small.tile([128, 1], dt, tag="nth")
                            nc.scalar.activation(
                                nth[:, :], t24[:, 0:1], AF.Identity, scale=-1.0
                            )
                            e20 = small.tile([128, 20], dt, tag="e20")
                            z = small.tile([128, 1], dt, tag="z")
                            nc.scalar.activation(
                                e20[:, :],
                                t24[:, 0:20],
                                AF.Exp,
                                bias=nth[:, :],
                                accum_out=z[:, :],
                            )
                            lnz = small.tile([128, 1], dt, tag="lnz")
                            nc.scalar.activation(lnz[:, :], z[:, :], AF.Ln)
                            bias2 = small.tile([128, 1], dt, tag="bias2")
                            nc.scalar.activation(
                                bias2[:, :],
                                lnz[:, :],
                                AF.Identity,
                                scale=-1.0,
                                bias=nth[:, :],
                            )
                            # e = exp(score - mx - lnz) in fp16
                            e = att_sb.tile([128, N], fp16, tag="e")
                            nc.scalar.activation(
                                e[:, :], score[:, :], AF.Exp, bias=bias2[:, :]
                            )
                            # W = (score >= th) * e -> fp16 (compare on fp32)
                            wh = att_sb.tile([128, N], fp16, tag="wh")
                            nc.vector.scalar_tensor_tensor(
                                wh[:, :],
                                score[:, :],
                                th,
                                e[:, :],
                                op0=ALU.is_ge,
                                op1=ALU.mult,
                            )
                            # W^T via whole-row DMA transpose (2B dtype)
                            nc.sync.dma_start_transpose(
                                wt4[:, :, s, :],
                                wh[:, :],
                            )
                        # agg^T[d, n] = sum_j fT[j, d] * WT[j, n] over the group
                        gsl = slice(g * GRP * 128, (g + 1) * GRP * 128)
                        pa = ps_a.tile([Cf, GRP * 128], dt, tag="pa")
                        for j in range(NCH):
                            nc.tensor.matmul(
                                pa[:, :],
                                fT[:, j, 0:Cf],
                                wt4[:, j, :, :],
                                start=(j == 0),
                                stop=(j == NCH - 1),
                            )
                        nc.vector.tensor_sub(agg_dst[:, gsl], pa[:, :], f_src[:, gsl])

            def conv_bn_lrelu(chunks, li, dests):
                """chunks: list of (fp16 ap [Ck, N], Ck). dests: fp16 [coP, N]."""
                C_in, C_out = convs[li]
                coP = min(C_out, 128)
                CO = (C_out + 127) // 128
                KC = len(chunks)
                assert sum(c for _, c in chunks) == C_in
                wt = wt_sb[li]
                with (
                    tc.tile_pool(name=f"conv{li}", bufs=1) as cp,
                    tc.tile_pool(name=f"convps{li}", bufs=3, space="PSUM") as cps,
                ):
                    y_sb = cp.tile([coP, CO, N], fp16)
                    st = cp.tile([coP, CO, 2], dt)
                    sum_parts = cp.tile([coP, CO, NB], dt)
                    for co in range(CO):
                        for nb in range(NB):
                            ps = cps.tile([128, 512], dt, tag="cps")
                            for kc, (cap, Ck) in enumerate(chunks):
                                nc.tensor.matmul(
                                    ps[0:coP, :],
                                    wt[0:Ck, kc, co, :],
                                    cap[:, nb * 512 : (nb + 1) * 512],
                                    start=(kc == 0),
                                    stop=(kc == KC - 1),
                                )
                            nc.scalar.activation(
                                y_sb[:, co, nb * 512 : (nb + 1) * 512],
                                ps[0:coP, :],
                                AF.Copy,
                                accum_out=sum_parts[:, co, nb : nb + 1],
                            )
                        nc.scalar.activation(
                            scratch_h[0:coP, :],
                            y_sb[:, co, :],
                            AF.Square,
                            accum_out=st[:, co, 1:2],
                        )
                    nc.vector.reduce_sum(st[:, :, 0:1], sum_parts[:, :, :], axis=AX.X)
                    # AllReduce stats across the 8 cores
                    st_in = dram.tile([coP, CO * 2], dt, tag=f"cc_in{li}")
                    st_out = dram.tile([coP, CO * 2], dt, tag=f"cc_out{li}")
                    nc.sync.dma_start(st_in[:, :], st[:, :, :])
                    nc.gpsimd.collective_compute(
                        "AllReduce",
                        ALU.add,
                        replica_groups=[list(range(8))],
                        ins=[st_in[:, :]],
                        outs=[st_out[:, :]],
                    )
                    gst = cp.tile([coP, CO, 2], dt)
                    nc.sync.dma_start(gst[:, :, :], st_out[:, :])
                    # scale/shift from global stats
                    m = cp.tile([coP, CO], dt)
                    ex2 = cp.tile([coP, CO], dt)
                    var = cp.tile([coP, CO], dt)
                    rstd = cp.tile([coP, CO], dt)
                    sc = cp.tile([coP, CO], dt)
                    sh = cp.tile([coP, CO], dt)
                    gg = cp.tile([coP, CO], dt)
                    bb = cp.tile([coP, CO], dt)
                    nc.sync.dma_start(
                        gg[:, :], g_d[li].ap().rearrange("(a p) -> p a", p=coP)
                    )
                    nc.sync.dma_start(
                        bb[:, :], b_d[li].ap().rearrange("(a p) -> p a", p=coP)
                    )
                    nc.vector.tensor_scalar_mul(m[:, :], gst[:, :, 0], 1.0 / BN_CNT)
                    nc.vector.tensor_scalar_mul(ex2[:, :], gst[:, :, 1], 1.0 / BN_CNT)
                    nc.vector.tensor_mul(var[:, :], m[:, :], m[:, :])
                    nc.vector.tensor_sub(var[:, :], ex2[:, :], var[:, :])
                    # rstd = (var+eps)^-0.5 = exp(-0.5*ln(var+eps))
                    nc.scalar.activation(
                        rstd[:, :], var[:, :], AF.Ln, bias=eps_t[0:coP, :]
                    )
                    nc.scalar.activation(rstd[:, :], rstd[:, :], AF.Exp, scale=-0.5)
                    nc.vector.tensor_mul(sc[:, :], gg[:, :], rstd[:, :])
                    nc.vector.tensor_mul(sh[:, :], m[:, :], sc[:, :])
                    nc.vector.tensor_sub(sh[:, :], bb[:, :], sh[:, :])
                    for co in range(CO):
                        nc.scalar.activation(
                            dests[co],
                            y_sb[:, co, :],
                            AF.Prelu,
                            bias=sh[:, co : co + 1],
                            scale=sc[:, co : co + 1],
                            alpha=0.01,
                        )

            def self_att_layer(attR, attL, C, li, dests, next_copy=None):
                """attR rows 0:C hold fp16 x; builds aug rows + attL, runs
                attention + conv. dests: fp16 prelu outputs."""
                K_dim = C + 2
                # attL = [2x ; ones ; ones]
                nc.vector.tensor_scalar_mul(attL[0:C, :], attR[0:C, :], 2.0)
                nc.sync.dma_start(attL[C : C + 2, :], ones2h[:, :])
                # attR aug rows = fp16 pair of -xx
                make_negxx_pair(attR[0:C, :], C, attR[C : C + 2, :])
                with tc.tile_pool(name=f"sa{li}", bufs=1) as sp:
                    xT = sp.tile([128, NCH, C + (C % 2)], fp16)
                    transpose_to(attR[0:C, :], C, xT)
                    agg = sp.tile([C, N], fp16)
                    attention(
                        lambda isl: attL[0 : C + 2, isl],
                        K_dim,
                        C,
                        xT,
                        attR[0:C, :],
                        None,
                        lambda nb: attR[0 : C + 2, nb * 512 : (nb + 1) * 512],
                        agg,
                    )
                    conv_bn_lrelu([(attR[0:C, :], C), (agg[:, :], C)], li, dests)
                if next_copy is not None:
                    src, dst = next_copy
                    nc.sync.dma_start(dst, src)

            # ---------------- Layer 1 (fp32 scores over 3-D points) --------
            nc.sync.dma_start(attR1[0:3, :], x_d[:, :])
            nc.scalar.copy(xh1[:, :], attR1[0:3, :])
            nc.vector.tensor_scalar_mul(attL1[0:3, :], attR1[0:3, :], 2.0)
            nc.sync.dma_start(attL1[3:4, :], ones_row32[:, :])
            col_sumsq_row(attR1[0:3, :], 3, xxrow)
            nc.scalar.activation(negxx32[0:1, :], xxrow[0:1, :], AF.Copy, scale=-1.0)
            nc.sync.dma_start(attR1[3:4, :], negxx32[0:1, :])
            with tc.tile_pool(name="sa1", bufs=1) as sp1:
                xT1 = sp1.tile([128, NCH, 4], fp16)
                transpose_to(xh1[:, :], 3, xT1)
                agg1 = sp1.tile([3, N], fp16)
                attention(
                    lambda isl: attL1[0:4, isl],
                    4,
                    3,
                    xT1,
                    xh1[:, :],
                    None,
                    lambda nb: attR1[0:4, nb * 512 : (nb + 1) * 512],
                    agg1,
                )
                conv_bn_lrelu(
                    [(xh1[:, :], 3), (agg1[:, :], 3)], 1, [attR2[0:64, :]]
                )
            nc.sync.dma_start(xc[0:64, 0, :], attR2[0:64, :])
            # ---------------- Layer 2 ----------------
            self_att_layer(
                attR2,
                attL2,
                64,
                2,
                [attR3[0:64, :]],
                next_copy=(attR3[0:64, :], xc[64:128, 0, :]),
            )
            # ---------------- Layer 3 ----------------
            self_att_layer(attR3, attL3, 64, 3, [xc[:, 1, :]])
            # ---------------- Layer 4 (cross) ----------------
            x3 = xc[:, 1, :]
            with tc.tile_pool(name="ca", bufs=1) as ca:
                y3_h = ca.tile([128, N], fp16)
                nc.gpsimd.dma_start(y3_h[:, :], y3_d[:, :])
                # rnx (per-row 1/||x3_n||) in column form
                xx3 = ca.tile([1, N], dt)
                col_sumsq_row(x3, 128, xx3)
                xx3c = ca.tile([128, NCH], dt)
                with tc.tile_pool(name="psr", bufs=2, space="PSUM") as psr:
                    ps = psr.tile([128, NCH], dt)
                    for t in range(NCH):
                        nc.tensor.transpose(
                            ps[:, t : t + 1],
                            xx3[0:1, t * 128 : (t + 1) * 128],
                            ident[0:1, 0:1],
                        )
                    nc.scalar.copy(xx3c[:, :], ps[:, :])
                rnxc = ca.tile([128, NCH], dt)
                nc.scalar.activation(rnxc[:, :], xx3c[:, :], AF.Ln)
                nc.scalar.activation(rnxc[:, :], rnxc[:, :], AF.Exp, scale=-0.5)
                # rny (per-col 1/||y_m||) in row form; yn = y3 * rny
                yy = ca.tile([1, N], dt)
                col_sumsq_row(y3_h[:, :], 128, yy)
                rny = ca.tile([1, N], fp16)
                nc.scalar.activation(yy[:, :], yy[:, :], AF.Ln)
                nc.scalar.activation(rny[:, :], yy[:, :], AF.Exp, scale=-0.5)
                rnyb = ca.tile([128, N], fp16)
                with tc.tile_pool(name="psb", bufs=2, space="PSUM") as psb:
                    for nb in range(NB):
                        ps = psb.tile([128, 512], dt)
                        nc.tensor.matmul(
                            ps[:, :],
                            ones2h[0:1, 0:128],
                            rny[0:1, nb * 512 : (nb + 1) * 512],
                            start=True,
                            stop=True,
                        )
                        nc.scalar.copy(rnyb[:, nb * 512 : (nb + 1) * 512], ps[:, :])
                yn_h = ca.tile([128, N], fp16)
                nc.vector.tensor_mul(yn_h[:, :], y3_h[:, :], rnyb[:, :])
                y3T = ca.tile([128, NCH, 128], fp16)
                transpose_to(y3_h[:, :], 128, y3T)
                agg4 = ca.tile([128, N], fp16)
                attention(
                    lambda isl: x3[:, isl],
                    128,
                    128,
                    y3T,
                    x3,
                    rnxc,
                    lambda nb: yn_h[:, nb * 512 : (nb + 1) * 512],
                    agg4,
                )
                conv_bn_lrelu(
                    [(x3, 128), (agg4[:, :], 128)], 4, [xc[:, 2, :], xc[:, 3, :]]
                )
            # ---------------- Layer 5 ----------------
            with tc.tile_pool(name="l5", bufs=1) as l5:
                out_sb = l5.tile([128, 4, N], dt)
                conv_bn_lrelu(
                    [(xc[:, c, :], 128) for c in range(4)],
                    5,
                    [out_sb[:, c, :] for c in range(4)],
                )
                nc.sync.dma_start(
                    out_d.ap().rearrange("(a p) n -> p a n", p=128), out_sb[:, :, :]
                )

    nc.finalize()
    return nc


def kernel(**inputs):
    if "nc" not in _CACHE:
        _CACHE["nc"] = _build()
    nc = _CACHE["nc"]
    from concourse.bass_utils import run_bass_kernel_spmd

    names = ["w1", "w2", "w3", "w4", "w5"] + [
        f"{p}{i}" for i in range(1, 6) for p in ("g", "b")
    ]
    in_maps = []
    for b in range(B):
        m = {
            "x": np.ascontiguousarray(inputs["x"][b]),
            "y3": np.ascontiguousarray(inputs["y3"][b]),
        }
        for k in names:
            m[k] = np.ascontiguousarray(inputs[k])
        in_maps.append(m)
    res = run_bass_kernel_spmd(nc, in_maps, core_ids=list(range(B)))
    return np.stack([res.results[b]["out"] for b in range(B)])


# revision 5
# speedup vs baseline: 1.1538x; 1.1538x over previous
"""AGCNN_IA Trainium2 kernel v2: 3x self-att + 1x cross-att + 5x conv-BN-lrelu.

Sharding: data-parallel over batch B=8 across 8 NeuronCores (1 sample/core).
BN batch statistics are AllReduce'd (sum, sumsq per channel) across cores.

v2 redesign vs v1 (1.31 ms):
  - features kept in fp16 everywhere (prelu writes fp16); self-att scores are
    computed EXACTLY over the fp16 feature values by a single fp16 matmul
    (fp16 products accumulate in fp32 on the PE), with the -xx[m] column term
    riding as two extra fp16 rows (hi/lo pair) of the rhs against ones rows
    in lhsT. This replaces v1's 4-cycle fp32 matmuls (4x PE reduction).
  - top-20 via 16 segment max8 (top-8 per 128-wide segment -> 128 candidates)
    + 3 max8/2 match_replace on the candidates. Replaces 5 full-width DVE
    passes (11.4 us/chunk) with ~1.2 full passes (~5 us/chunk).
  - threshold mask (score >= t20) evaluated on exact fp32 scores via one DVE
    scalar_tensor_tensor; weights = mask * exp(score - mx - ln z20).
  - convs all fp16 (weights transposed to fp16), BN stats fp32 accum.
"""

import sys

import numpy as np

sys.path.insert(0, "/opt/trn_rl_repo")

B = 8
N = 2048
KTOP = 20
EPS_BN = 1e-5
NCH = N // 128  # 16 row chunks
NB = N // 512  # 4 matmul free-dim blocks
NSEG = 16  # topk segments per row chunk
SEGW = N // NSEG
BN_CNT = float(B * N)

_CACHE = {}


def _patch_act_tables():
    """Force every ACT function onto the one table set that has them all."""
    from concourse import bacc

    orig = bacc.get_activation_tables
    if getattr(orig, "_patched_single_set", False):
        return

    def patched(module_arch):
        tables = orig(module_arch)
        keep = "natural_log_exp_and_others"
        if keep in tables:
            tables = {
                name: (fns if name == keep else set())
                for name, fns in tables.items()
            }
        return tables

    patched._patched_single_set = True
    bacc.get_activation_tables = patched


def _build():
    import concourse.bass as bass
    import concourse.mybir as mybir
    from concourse import bacc, tile
    from concourse.masks import make_identity

    _patch_act_tables()

    dt = mybir.dt.float32
    fp16 = mybir.dt.float16
    AF = mybir.ActivationFunctionType
    ALU = mybir.AluOpType
    AX = mybir.AxisListType

    nc = bacc.Bacc(None, target_bir_lowering=False, debug=False, num_devices=8)

    x_d = nc.declare_dram_parameter("x", [3, N], dt, isOutput=False)
    y3_d = nc.declare_dram_parameter("y3", [128, N], dt, isOutput=False)
    w_d = {}
    g_d = {}
    b_d = {}
    convs = {1: (6, 64), 2: (128, 64), 3: (128, 128), 4: (256, 256), 5: (512, 512)}
    for i, (ci, co) in convs.items():
        w_d[i] = nc.declare_dram_parameter(f"w{i}", [co, ci], dt, isOutput=False)
        g_d[i] = nc.declare_dram_parameter(f"g{i}", [co], dt, isOutput=False)
        b_d[i] = nc.declare_dram_parameter(f"b{i}", [co], dt, isOutput=False)
    out_d = nc.declare_dram_parameter("out", [512, N], dt, isOutput=True)

    with tile.TileContext(nc) as tc:
        with (
            tc.tile_pool(name="persist", bufs=1) as persist,
            tc.tile_pool(name="scratch", bufs=1) as scratch_pool,
            tc.tile_pool(name="dram", bufs=1, space="DRAM") as dram,
        ):
            ident = persist.tile([128, 128], dt)
            make_identity(nc, ident[:, :])
            ident_h = persist.tile([128, 128], fp16)
            make_identity(nc, ident_h[:, :])
            ones_col_h = persist.tile([128, 1], fp16)
            nc.vector.memset(ones_col_h[:, :], 1.0)
            ones2h = persist.tile([2, N], fp16)
            nc.vector.memset(ones2h[:, :], 1.0)
            eps_t = persist.tile([128, 1], dt)
            nc.vector.memset(eps_t[:, :], EPS_BN)
            # xc: concat buffer [128, 4, N] fp16; ch c = xc[c%128, c//128, :]
            xc = persist.tile([128, 4, N], fp16)
            # attention in/out feature tiles (fp16): attR holds [x ; negxx pair]
            # L1 (3-D points) is fp32: NN selection there is hypersensitive to
            # coordinate rounding, so scores/xx stay exact fp32.
            attR1 = persist.tile([4, N], dt)
            attL1 = persist.tile([4, N], dt)
            xh1 = persist.tile([3, N], fp16)
            ones_row32 = persist.tile([1, N], dt)
            nc.vector.memset(ones_row32[:, :], 1.0)
            ones_col32 = persist.tile([128, 1], dt)
            nc.vector.memset(ones_col32[:, :], 1.0)
            scratch32 = persist.tile([128, N], dt)
            attR2 = persist.tile([66, N], fp16)
            attL2 = persist.tile([66, N], fp16)
            attR3 = persist.tile([66, N], fp16)
            attL3 = persist.tile([66, N], fp16)
            # fp16 elementwise scratch (squares for sumsq, conv square dump)
            scratch_h = scratch_pool.tile([128, N], fp16)
            xxrow = persist.tile([1, N], dt)
            negxx32 = persist.tile([1, N], dt)
            pair_hi = persist.tile([1, N], fp16)
            pair_lo = persist.tile([1, N], fp16)

            # ---------- conv weight prep (no data deps; scheduler overlaps) --
            wt_sb = {}
            with (
                tc.tile_pool(name="wprep", bufs=1) as wp,
                tc.tile_pool(name="wprep_ps", bufs=2, space="PSUM") as wps,
            ):
                chunk_splits = {
                    1: [3, 3],
                    2: [64, 64],
                    3: [64, 64],
                    4: [128, 128],
                    5: [128, 128, 128, 128],
                }
                for li, (C_in, C_out) in convs.items():
                    coP = min(C_out, 128)
                    CO = (C_out + 127) // 128
                    splits = chunk_splits[li]
                    w_sb = wp.tile([coP, CO, C_in], dt, tag=f"w_sb{li}")
                    nc.sync.dma_start(
                        w_sb[:, :, :],
                        w_d[li].ap().rearrange("(a p) c -> p a c", p=coP),
                    )
                    wt = persist.tile(
                        [128, len(splits), CO, coP], fp16, tag=f"wt{li}"
                    )
                    off = 0
                    for kc, Ck in enumerate(splits):
                        for co in range(CO):
                            ps = wps.tile([128, 128], dt, tag="wtps")
                            nc.tensor.transpose(
                                ps[0:Ck, 0:coP],
                                w_sb[0:coP, co, off : off + Ck],
                                ident[0:coP, 0:coP],
                            )
                            nc.scalar.copy(wt[0:Ck, kc, co, :], ps[0:Ck, 0:coP])
                        off += Ck
                    wt_sb[li] = wt

            def col_sumsq_row(src_ap, C, dst_row_ap):
                """dst_row[0, m] = sum_c src[c, m]^2 (fp32 squares + fp32 PE)."""
                nc.scalar.activation(scratch32[0:C, :], src_ap, AF.Square)
                with tc.tile_pool(name="psx", bufs=2, space="PSUM") as psx:
                    for nb in range(NB):
                        ps = psx.tile([1, 512], dt)
                        nc.tensor.matmul(
                            ps[:, :],
                            ones_col32[0:C, :],
                            scratch32[0:C, nb * 512 : (nb + 1) * 512],
                            start=True,
                            stop=True,
                        )
                        nc.scalar.copy(
                            dst_row_ap[0:1, nb * 512 : (nb + 1) * 512], ps[:, :]
                        )

            def make_negxx_pair(src_ap, C, dst_rows_ap):
                """attR aug rows: fp16 hi/lo pair of -xx, DMA'd into place."""
                col_sumsq_row(src_ap, C, xxrow)
                nc.scalar.activation(
                    pair_hi[0:1, :], xxrow[0:1, :], AF.Copy, scale=-1.0
                )
                # lo = (-xx) - hi
                nc.vector.scalar_tensor_tensor(
                    pair_lo[0:1, :],
                    xxrow[0:1, :],
                    -1.0,
                    pair_hi[0:1, :],
                    op0=ALU.mult,
                    op1=ALU.subtract,
                )
                nc.sync.dma_start(dst_rows_ap[0:1, :], pair_hi[0:1, :])
                nc.sync.dma_start(dst_rows_ap[1:2, :], pair_lo[0:1, :])

            def transpose_to(src_ap, C, dst_tile):
                """src [C, N] fp16 -> dst [128, NCH, >=C] fp16."""
                Cp = C + (C % 2)  # 4B-aligned PSUM rows for fp16
                with tc.tile_pool(name="pst", bufs=2, space="PSUM") as pst:
                    for t0 in range(0, NCH, 4):
                        ps = pst.tile([128, 4, Cp], fp16)
                        for t in range(t0, t0 + 4):
                            nc.tensor.transpose(
                                ps[:, t - t0, 0:C],
                                src_ap[:, t * 128 : (t + 1) * 128],
                                ident_h[0:C, 0:C],
                            )
                        nc.scalar.copy(
                            dst_tile[:, t0 : t0 + 4, 0:C], ps[:, :, 0:C]
                        )

            def attention(lhsT_ap, K_dim, Cf, fT, f_src, rnx_col, get_rhs, agg_dst):
                """Top-k attention. agg_dst [Cf, N] fp16 = W @ f - f_src.

                lhsT_ap(isl) -> [K_dim, 128] fp16 stationary for score matmul
                get_rhs(nb) -> [K_dim, 512] fp16 moving
                fT: [128, NCH, Cf] fp16 transposed gather-source features
                f_src: [Cf, N] fp16 subtract source
                rnx_col: [128, NCH] per-row 1/||x_n|| or None
                """
                with (
                    tc.tile_pool(name="att_sb", bufs=2) as att_sb,
                    tc.tile_pool(name="att_w", bufs=2) as att_w,
                    tc.tile_pool(name="att_small", bufs=2) as small,
                    tc.tile_pool(name="ps_g", bufs=4, space="PSUM") as ps_g,
                    tc.tile_pool(name="ps_a", bufs=2, space="PSUM") as ps_a,
                ):
                    GRP = 2
                    for g in range(NCH // GRP):  # groups of row-chunks
                        wt4 = att_w.tile([128, NCH, GRP, 128], fp16, tag="wt4")
                        for s in range(GRP):  # sub-chunk within group
                            i = g * GRP + s
                            isl = slice(i * 128, (i + 1) * 128)
                            score = att_sb.tile([128, N], dt, tag="score")
                            for nb in range(NB):
                                ps = ps_g.tile([128, 512], dt, tag="ps")
                                nc.tensor.matmul(
                                    ps[:, :],
                                    lhsT_ap(isl),
                                    get_rhs(nb),
                                    start=True,
                                    stop=True,
                                )
                                if rnx_col is None:
                                    nc.scalar.copy(
                                        score[:, nb * 512 : (nb + 1) * 512],
                                        ps[:, :],
                                    )
                                else:
                                    nc.scalar.activation(
                                        score[:, nb * 512 : (nb + 1) * 512],
                                        ps[:, :],
                                        AF.Copy,
                                        scale=rnx_col[:, i : i + 1],
                                    )
                            # segment top-8 candidates (exact unless one
                            # segment holds >8 of the true top-20: ~1e-5)
                            cand = small.tile([128, NSEG * 8], dt, tag="cand")
                            for sg in range(NSEG):
                                nc.vector.max(
                                    cand[:, sg * 8 : (sg + 1) * 8],
                                    score[:, sg * SEGW : (sg + 1) * SEGW],
                                )
                            t24 = small.tile([128, 32], dt, tag="t24")
                            candm = small.tile([128, NSEG * 8], dt, tag="candm")
                            nc.vector.max(t24[:, 0:8], cand[:, :])
                            nc.vector.match_replace(
                                candm[:, :], t24[:, 0:8], cand[:, :], -1e30
                            )
                            nc.vector.max(t24[:, 8:16], candm[:, :])
                            nc.vector.match_replace(
                                candm[:, :], t24[:, 8:16], candm[:, :], -1e30
                            )
                            nc.vector.max(t24[:, 16:24], candm[:, :])
                            th = t24[:, 19:20]
                            # bias2 = -(row_max + ln z20)
                            nth = small.tile([128, 1], dt, tag="nth")
                            nc.scalar.activation(
                                nth[:, :], t24[:, 0:1], AF.Identity, scale=-1.0
                            )
                            e20 = small.tile([128, 20], dt, tag="e20")
                            z = small.tile([128, 1], dt, tag="z")
                            nc.scalar.activation(
                                e20[:, :],
                                t24[:, 0:20],
                                AF.Exp,
                                bias=nth[:, :],
                                accum_out=z[:, :],
                            )
                            lnz = small.tile([128, 1], dt, tag="lnz")
                            nc.scalar.activation(lnz[:, :], z[:, :], AF.Ln)
                            bias2 = small.tile([128, 1], dt, tag="bias2")
                            nc.scalar.activation(
                                bias2[:, :],
                                lnz[:, :],
                                AF.Identity,
                                scale=-1.0,
                                bias=nth[:, :],
                            )
                            # e = exp(score - mx - lnz) in fp16
                            e = att_sb.tile([128, N], fp16, tag="e")
                            nc.scalar.activation(
                                e[:, :], score[:, :], AF.Exp, bias=bias2[:, :]
                            )
                            # W = (score >= th) * e -> fp16 (compare on fp32)
                            wh = att_sb.tile([128, N], fp16, tag="wh")
                            nc.vector.scalar_tensor_tensor(
                                wh[:, :],
                                score[:, :],
                                th,
                                e[:, :],
                                op0=ALU.is_ge,
                                op1=ALU.mult,
                            )
                            # W^T via whole-row DMA transpose (2B dtype)
                            nc.sync.dma_start_transpose(
                                wt4[:, :, s, :],
                                wh[:, :],
                            )
                        # agg^T[d, n] = sum_j fT[j, d] * WT[j, n] over the group
                        gsl = slice(g * GRP * 128, (g + 1) * GRP * 128)
                        pa = ps_a.tile([Cf, GRP * 128], dt, tag="pa")
                        for j in range(NCH):
                            nc.tensor.matmul(
                                pa[:, :],
                                fT[:, j, 0:Cf],
                                wt4[:, j, :, :],
                                start=(j == 0),
                                stop=(j == NCH - 1),
                            )
                        nc.vector.tensor_sub(agg_dst[:, gsl], pa[:, :], f_src[:, gsl])

            def conv_bn_lrelu(chunks, li, dests):
                """chunks: list of (fp16 ap [Ck, N], Ck). dests: fp16 [coP, N]."""
                C_in, C_out = convs[li]
                coP = min(C_out, 128)
                CO = (C_out + 127) // 128
                KC = len(chunks)
                assert sum(c for _, c in chunks) == C_in
                wt = wt_sb[li]
                with (
                    tc.tile_pool(name=f"conv{li}", bufs=1) as cp,
                    tc.tile_pool(name=f"convps{li}", bufs=3, space="PSUM") as cps,
                ):
                    y_sb = cp.tile([coP, CO, N], fp16)
                    st = cp.tile([coP, CO, 2], dt)
                    sum_parts = cp.tile([coP, CO, NB], dt)
                    for co in range(CO):
                        for nb in range(NB):
                            ps = cps.tile([128, 512], dt, tag="cps")
                            for kc, (cap, Ck) in enumerate(chunks):
                                nc.tensor.matmul(
                                    ps[0:coP, :],
                                    wt[0:Ck, kc, co, :],
                                    cap[:, nb * 512 : (nb + 1) * 512],
                                    start=(kc == 0),
                                    stop=(kc == KC - 1),
                                )
                            nc.scalar.activation(
                                y_sb[:, co, nb * 512 : (nb + 1) * 512],
                                ps[0:coP, :],
                                AF.Copy,
                                accum_out=sum_parts[:, co, nb : nb + 1],
                            )
                        nc.scalar.activation(
                            scratch_h[0:coP, :],
                            y_sb[:, co, :],
                            AF.Square,
                            accum_out=st[:, co, 1:2],
                        )
                    nc.vector.reduce_sum(st[:, :, 0:1], sum_parts[:, :, :], axis=AX.X)
                    # AllReduce stats across the 8 cores
                    st_in = dram.tile([coP, CO * 2], dt, tag=f"cc_in{li}")
                    st_out = dram.tile([coP, CO * 2], dt, tag=f"cc_out{li}")
                    nc.sync.dma_start(st_in[:, :], st[:, :, :])
                    nc.gpsimd.collective_compute(
                        "AllReduce",
                        ALU.add,
                        replica_groups=[list(range(8))],
                        ins=[st_in[:, :]],
                        outs=[st_out[:, :]],
                    )
                    gst = cp.tile([coP, CO, 2], dt)
                    nc.sync.dma_start(gst[:, :, :], st_out[:, :])
                    # scale/shift from global stats
                    m = cp.tile([coP, CO], dt)
                    ex2 = cp.tile([coP, CO], dt)
                    var = cp.tile([coP, CO], dt)
                    rstd = cp.tile([coP, CO], dt)
                    sc = cp.tile([coP, CO], dt)
                    sh = cp.tile([coP, CO], dt)
                    gg = cp.tile([coP, CO], dt)
                    bb = cp.tile([coP, CO], dt)
                    nc.sync.dma_start(
                        gg[:, :], g_d[li].ap().rearrange("(a p) -> p a", p=coP)
                    )
                    nc.sync.dma_start(
                        bb[:, :], b_d[li].ap().rearrange("(a p) -> p a", p=coP)
                    )
                    nc.vector.tensor_scalar_mul(m[:, :], gst[:, :, 0], 1.0 / BN_CNT)
                    nc.vector.tensor_scalar_mul(ex2[:, :], gst[:, :, 1], 1.0 / BN_CNT)
                    nc.vector.tensor_mul(var[:, :], m[:, :], m[:, :])
                    nc.vector.tensor_sub(var[:, :], ex2[:, :], var[:, :])
                    # rstd = (var+eps)^-0.5 = exp(-0.5*ln(var+eps))
                    nc.scalar.activation(
                        rstd[:, :], var[:, :], AF.Ln, bias=eps_t[0:coP, :]
                    )
                    nc.scalar.activation(rstd[:, :], rstd[:, :], AF.Exp, scale=-0.5)
                    nc.vector.tensor_mul(sc[:, :], gg[:, :], rstd[:, :])
                    nc.vector.tensor_mul(sh[:, :], m[:, :], sc[:, :])
                    nc.vector.tensor_sub(sh[:, :], bb[:, :], sh[:, :])
                    for co in range(CO):
                        nc.scalar.activation(
                            dests[co],
                            y_sb[:, co, :],
                            AF.Prelu,
                            bias=sh[:, co : co + 1],
                            scale=sc[:, co : co + 1],
                            alpha=0.01,
                        )

            def self_att_layer(attR, attL, C, li, dests, next_copy=None):
                """attR rows 0:C hold fp16 x; builds aug rows + attL, runs
                attention + conv. dests: fp16 prelu outputs."""
                K_dim = C + 2
                # attL = [2x ; ones ; ones]
                nc.vector.tensor_scalar_mul(attL[0:C, :], attR[0:C, :], 2.0)
                nc.sync.dma_start(attL[C : C + 2, :], ones2h[:, :])
                # attR aug rows = fp16 pair of -xx
                make_negxx_pair(attR[0:C, :], C, attR[C : C + 2, :])
                with tc.tile_pool(name=f"sa{li}", bufs=1) as sp:
                    xT = sp.tile([128, NCH, C + (C % 2)], fp16)
                    transpose_to(attR[0:C, :], C, xT)
                    agg = sp.tile([C, N], fp16)
                    attention(
                        lambda isl: attL[0 : C + 2, isl],
                        K_dim,
                        C,
                        xT,
                        attR[0:C, :],
                        None,
                        lambda nb: attR[0 : C + 2, nb * 512 : (nb + 1) * 512],
                        agg,
                    )
                    conv_bn_lrelu([(attR[0:C, :], C), (agg[:, :], C)], li, dests)
                if next_copy is not None:
                    src, dst = next_copy
                    nc.sync.dma_start(dst, src)

            # ---------------- Layer 1 (fp32 scores over 3-D points) --------
            nc.sync.dma_start(attR1[0:3, :], x_d[:, :])
            nc.scalar.copy(xh1[:, :], attR1[0:3, :])
            nc.vector.tensor_scalar_mul(attL1[0:3, :], attR1[0:3, :], 2.0)
            nc.sync.dma_start(attL1[3:4, :], ones_row32[:, :])
            col_sumsq_row(attR1[0:3, :], 3, xxrow)
            nc.scalar.activation(negxx32[0:1, :], xxrow[0:1, :], AF.Copy, scale=-1.0)
            nc.sync.dma_start(attR1[3:4, :], negxx32[0:1, :])
            with tc.tile_pool(name="sa1", bufs=1) as sp1:
                xT1 = sp1.tile([128, NCH, 4], fp16)
                transpose_to(xh1[:, :], 3, xT1)
                agg1 = sp1.tile([3, N], fp16)
                attention(
                    lambda isl: attL1[0:4, isl],
                    4,
                    3,
                    xT1,
                    xh1[:, :],
                    None,
                    lambda nb: attR1[0:4, nb * 512 : (nb + 1) * 512],
                    agg1,
                )
                conv_bn_lrelu(
                    [(xh1[:, :], 3), (agg1[:, :], 3)], 1, [attR2[0:64, :]]
                )
            nc.sync.dma_start(xc[0:64, 0, :], attR2[0:64, :])
            # ---------------- Layer 2 ----------------
            self_att_layer(
                attR2,
                attL2,
                64,
                2,
                [attR3[0:64, :]],
                next_copy=(attR3[0:64, :], xc[64:128, 0, :]),
            )
            # ---------------- Layer 3 ----------------
            self_att_layer(attR3, attL3, 64, 3, [xc[:, 1, :]])
            # ---------------- Layer 4 (cross) ----------------
            x3 = xc[:, 1, :]
            with tc.tile_pool(name="ca", bufs=1) as ca:
                y3_h = ca.tile([128, N], fp16)
                nc.gpsimd.dma_start(y3_h[:, :], y3_d[:, :])
                # rnx (per-row 1/||x3_n||) in column form
                xx3 = ca.tile([1, N], dt)
                col_sumsq_row(x3, 128, xx3)
                xx3c = ca.tile([128, NCH], dt)
                with tc.tile_pool(name="psr", bufs=2, space="PSUM") as psr:
                    ps = psr.tile([128, NCH], dt)
                    for t in range(NCH):
                        nc.tensor.transpose(
                            ps[:, t : t + 1],
                            xx3[0:1, t * 128 : (t + 1) * 128],
                            ident[0:1, 0:1],
                        )
                    nc.scalar.copy(xx3c[:, :], ps[:, :])
                rnxc = ca.tile([128, NCH], dt)
                nc.scalar.activation(rnxc[:, :], xx3c[:, :], AF.Ln)
                nc.scalar.activation(rnxc[:, :], rnxc[:, :], AF.Exp, scale=-0.5)
                # rny (per-col 1/||y_m||) in row form; yn = y3 * rny
                yy = ca.tile([1, N], dt)
                col_sumsq_row(y3_h[:, :], 128, yy)
                rny = ca.tile([1, N], fp16)
                nc.scalar.activation(yy[:, :], yy[:, :], AF.Ln)
                nc.scalar.activation(rny[:, :], yy[:, :], AF.Exp, scale=-0.5)
                rnyb = ca.tile([128, N], fp16)
                with tc.tile_pool(name="psb", bufs=2, space="PSUM") as psb:
                    for nb in range(NB):
                        ps = psb.tile([128, 512], dt)
                        nc.tensor.matmul(
                            ps[:, :],
                            ones2h[0:1, 0:128],
                            rny[0:1, nb * 512 : (nb + 1) * 512],
                            start=True,
                            stop=True,
                        )
                        nc.scalar.copy(rnyb[:, nb * 512 : (nb + 1) * 512], ps[:, :])
                yn_h = ca.tile([128, N], fp16)
                nc.vector.tensor_mul(yn_h[:, :], y3_h[:, :], rnyb[:, :])
                y3T = ca.tile([128, NCH, 128], fp16)
                transpose_to(y3_h[:, :], 128, y3T)
                agg4 = ca.tile([128, N], fp16)
                attention(
                    lambda isl: x3[:, isl],
                    128,
                    128,
                    y3T,
                    x3,
                    rnxc,
                    lambda nb: yn_h[:, nb * 512 : (nb + 1) * 512],
                    agg4,
                )
                conv_bn_lrelu(
                    [(x3, 128), (agg4[:, :], 128)], 4, [xc[:, 2, :], xc[:, 3, :]]
                )
            # ---------------- Layer 5 ----------------
            with tc.tile_pool(name="l5", bufs=1) as l5:
                out_sb = l5.tile([128, 4, N], dt)
                conv_bn_lrelu(
                    [(xc[:, c, :], 128) for c in range(4)],
                    5,
                    [out_sb[:, c, :] for c in range(4)],
                )
                nc.sync.dma_start(
                    out_d.ap().rearrange("(a p) n -> p a n", p=128), out_sb[:, :, :]
                )

    nc.finalize()
    return nc


def kernel(**inputs):
    if "nc" not in _CACHE:
        _CACHE["nc"] = _build()
    nc = _CACHE["nc"]
    from concourse.bass_utils import run_bass_kernel_spmd

    names = ["w1", "w2", "w3", "w4", "w5"] + [
        f"{p}{i}" for i in range(1, 6) for p in ("g", "b")
    ]
    in_maps = []
    for b in range(B):
        m = {
            "x": np.ascontiguousarray(inputs["x"][b]),
            "y3": np.ascontiguousarray(inputs["y3"][b]),
        }
        for k in names:
            m[k] = np.ascontiguousarray(inputs[k])
        in_maps.append(m)
    res = run_bass_kernel_spmd(nc, in_maps, core_ids=list(range(B)))
    return np.stack([res.results[b]["out"] for b in range(B)])
